# revision 1
# baseline (speedup 1.0000x reference)
"""Chamfer distance kernel for Trainium2 (8 NeuronCores, data-parallel over batch).

Problem: xyz1, xyz2: [8, 8192, 3] fp32.  Per batch b:
  d[i,j] = |x_i|^2 + |y_j|^2 - 2 x_i.y_j
  dist1[i] = min_j d[i,j]; idx1[i] = argmin_j d[i,j]   (and symmetrically dist2/idx2)

Strategy (one batch per core).  The device finds, per row i, the winning
128-wide COLUMN BLOCK of the argmax of e = -d, and per column j the winning
128-row TILE; the host then refines exactly (fp32) within the winning
128-candidate window and computes the dists from the indices, as in the
previous revision.  This removes all per-element index-tracking from the
device: no running-argmax tracker, no mask transposes, no index matmuls.

  - PE computes e = 2 x.y - sq1 - sq2 with a K=16 fp16 two-limb matmul
    (x = xh + xl, 2y = yh + yl in fp16; all four cross products plus
    two-limb -sq1 / -sq2 rows accumulate in fp32 PSUM).  fp16 limbs give
    ~1e-5 dot accuracy (vs fp32) at 1 cycle/row -- 4x faster than the fp32
    matmul datapath.  The limb rows are host-prepared (input marshalling).
  - ACT downconverts PSUM -> fp16 SBUF e_row in 2048-wide chunks.
  - Row path per tile: in-block max tree (DVE, 2x fp16) -> m64[p, blk],
    rmx = max(m64); mask64 = (m64 >= rmx) at 4x; one tensor_tensor_reduce
    against weights (4096 + blk) yields S1 = 4096*cnt + sum(winning blk):
    cnt==1 -> block index, else host recomputes the row.
  - Col path per tile: ONE batched xbar-transpose DMA of e_row ->
    et[jj, blk, i]; an i-max tree (first halving on GPSIMD, rest on DVE)
    yields the tile's per-column max, stored into cm[jj, blk, t].
  - Finals: max tree over t, compare, weighted add-tree -> S2 = 4096*cnt +
    sum(winning t) per column.  Host refines i within the winning tile.
"""

import numpy as np

B = 8
N = 8192  # rows per batch (xyz1 points)
M = 8192  # cols per batch (xyz2 points)
P = 128
K = 16    # matmul contraction rows (limb pairs + sq rows)
N_CORES = 8
OFF = 4096.0  # decode offset: S = OFF*cnt + sum(winner ids)

_cache = {}


def build(n=N, m=M, n_cores=N_CORES):
    """Build the Bass program. Returns the compiled Bacc object."""
    import concourse.bacc as bacc
    import concourse.tile as tile
    import concourse.mybir as mybir
    from concourse.bass_isa import ReduceOp

    dt = mybir.dt
    Alu = mybir.AluOpType

    nt = n // P   # row tiles
    nb = m // P   # column blocks
    assert nt <= 64 and nb <= 64
    # Column range [0, FP) gets its per-tile col-max via GPSIMD
    # partition_all_reduce (engine otherwise idle); [FP, m) goes through the
    # xbar transpose + DVE max tree.  FP balances Pool vs DVE busy time.
    FP = 5120
    nbp = FP // P          # pool-covered blocks
    nbt = nb - nbp         # tree-covered blocks

    nc = bacc.Bacc(
        "TRN2",
        target_bir_lowering=False,
        debug=False,
        enable_asserts=False,
        num_devices=n_cores,
    )

    lhs_d = nc.dram_tensor("lhs", [K, n], dt.float16, kind="ExternalInput").ap()
    rhs_d = nc.dram_tensor("rhs", [K, m], dt.float16, kind="ExternalInput").ap()
    rowres_d = nc.dram_tensor("rowres", [P, nt], dt.float32, kind="ExternalOutput").ap()
    colres_d = nc.dram_tensor("colres", [P, nb], dt.float32, kind="ExternalOutput").ap()

    with tile.TileContext(nc) as tc, tc.tile_pool(name="persist", bufs=1) as pp:
        lhs = pp.tile([K, n], dt.float16, name="lhs")
        rhs = pp.tile([K, m], dt.float16, name="rhs")
        # load the slices tile 0 needs first so its matmuls start early
        nc.sync.dma_start(lhs[:, 0:P], lhs_d[:, 0:P])
        nc.sync.dma_start(rhs[:, 0:2048], rhs_d[:, 0:2048])
        nc.sync.dma_start(rhs[:, 2048:m], rhs_d[:, 2048:m])
        nc.sync.dma_start(lhs[:, P:n], lhs_d[:, P:n])

        # weights wt64[p, b] = OFF + b (fp32, same on every partition)
        wt64 = pp.tile([P, nb], dt.float32, name="wt64")
        wtu = pp.tile([P, nb], dt.uint32, name="wtu")
        nc.gpsimd.iota(wtu[:], pattern=[[1, nb]], base=int(OFF), channel_multiplier=0)
        nc.vector.tensor_copy(wt64[:], wtu[:])

        # wtT[p, b, t] = OFF + t (zero-dep: built while the loop runs)
        wtT = pp.tile([P, nb, nt], dt.float32, name="wtT")
        wtTu = pp.tile([P, nb, nt], dt.uint32, name="wtTu")
        nc.gpsimd.iota(
            wtTu[:], pattern=[[0, nb], [1, nt]], base=int(OFF),
            channel_multiplier=0,
        )
        nc.vector.tensor_copy(wtT[:], wtTu[:])

        # per-tile column maxes: cm[jj, blk, t]
        cm = pp.tile([P, nb, nt], dt.float16, name="cm")
        # pool-path per-tile col maxes, row t = tile t (transposed at the end)
        cm2 = pp.tile([P, FP], dt.float16, name="cm2")
        nc.vector.memset(cm2[:], -60000.0)
        rowres_sb = pp.tile([P, nt], dt.float32, name="rowres_sb")
        colres_sb = pp.tile([P, nb], dt.float32, name="colres_sb")

        # ---- main loop ----
        with (
            tc.tile_pool(name="psum_e", bufs=2, space="PSUM") as psum_e,
            tc.tile_pool(name="e_pool", bufs=4) as e_pool,
            tc.tile_pool(name="et_pool", bufs=2) as et_pool,
            tc.tile_pool(name="scr_pool", bufs=2) as scr_pool,
            tc.tile_pool(name="isc_pool", bufs=2) as isc_pool,
            tc.tile_pool(name="px_pool", bufs=2) as px_pool,
            tc.tile_pool(name="tiny_pool", bufs=3) as tiny_pool,
        ):
            for t in range(nt):
                e_row = e_pool.tile([P, m], dt.float16, tag="e_row")
                for c in range(4):
                    ps = psum_e.tile([P, 2048], dt.float32, tag="ps")
                    for q in range(4):
                        nc.tensor.matmul(
                            ps[:, 512 * q:512 * (q + 1)],
                            lhs[:, t * P:(t + 1) * P],
                            rhs[:, c * 2048 + 512 * q: c * 2048 + 512 * (q + 1)],
                            start=True, stop=True,
                        )
                    nc.scalar.copy(e_row[:, c * 2048:(c + 1) * 2048], ps[:])

                # col path A (blocks 0..nbp): GPSIMD partition max, row-copy
                # the (replicated) result into partition t of cm2
                px = px_pool.tile([P, FP], dt.float16, tag="px")
                nc.gpsimd.partition_all_reduce(
                    px[:], e_row[:, 0:FP], P, ReduceOp.max
                )
                nc.sync.dma_start(cm2[t:t + 1, :], px[0:1, :])

                # col path B (blocks nbp..nb): transpose -> et[jj, blk, i],
                # then i-max tree
                et = et_pool.tile([P, nbt, P], dt.float16, tag="et")
                nc.sync.dma_start(et[:], e_row[:, FP:m], transpose=True)
                isc = isc_pool.tile([P, nbt, 64], dt.float16, tag="isc")
                nc.vector.tensor_max(isc[:], et[:, :, 0:64], et[:, :, 64:128])
                w = 32
                while w >= 2:
                    nc.vector.tensor_max(
                        isc[:, :, 0:w], isc[:, :, 0:w], isc[:, :, w:2 * w]
                    )
                    w //= 2
                nc.vector.tensor_max(
                    cm[:, nbp:nb, t:t + 1], isc[:, :, 0:1], isc[:, :, 1:2]
                )

                # row path: in-block max tree -> m64, rmx, mask64, TTR decode
                e3 = e_row[:].rearrange("p (b i) -> p b i", b=nb)
                scr = scr_pool.tile([P, nb, 64], dt.float16, tag="scr")
                h = nb // 2
                nc.vector.tensor_max(
                    scr[:, 0:h, :], e3[:, 0:h, 0:64], e3[:, 0:h, 64:128]
                )
                nc.vector.tensor_max(
                    scr[:, h:nb, :], e3[:, h:nb, 0:64], e3[:, h:nb, 64:128]
                )
                w = 32
                while w >= 2:
                    nc.vector.tensor_max(
                        scr[:, :, 0:w], scr[:, :, 0:w], scr[:, :, w:2 * w]
                    )
                    w //= 2
                m64 = tiny_pool.tile([P, nb], dt.float16, tag="m64")
                nc.vector.tensor_max(
                    m64[:].rearrange("p (b u) -> p b u", u=1),
                    scr[:, :, 0:1], scr[:, :, 1:2],
                )
                rmx = tiny_pool.tile([P, 1], dt.float32, tag="rmx")
                nc.vector.tensor_reduce(
                    rmx[:], m64[:], axis=mybir.AxisListType.X, op=Alu.max
                )
                mask64 = tiny_pool.tile([P, nb], dt.float16, tag="mask64")
                nc.vector.tensor_scalar(
                    mask64[:], m64[:], rmx[:], None, op0=Alu.is_ge
                )
                mw = tiny_pool.tile([P, nb], dt.float32, tag="mw")
                nc.vector.tensor_tensor(mw[:], mask64[:], wt64[:], op=Alu.mult)
                nc.vector.tensor_reduce(
                    rowres_sb[:, t:t + 1], mw[:], axis=mybir.AxisListType.X,
                    op=Alu.add,
                )

        # ---- finals: per-column argmax over tiles ----
        with tc.tile_pool(name="fin_pool", bufs=1) as fp:
            # fold the pool-path results into cm: transpose cm2[t, j] ->
            # cmT[jj, blk, tslot] and copy the valid tslots
            cmT = fp.tile([P, nbp, P], dt.float16, tag="cmT")
            nc.sync.dma_start(cmT[:], cm2[:], transpose=True)
            nc.vector.tensor_copy(cm[:, 0:nbp, :], cmT[:, :, 0:nt])

            ctr = fp.tile([P, nb, nt // 2], dt.float16, tag="ctr")
            nc.vector.tensor_max(ctr[:], cm[:, :, 0:nt // 2], cm[:, :, nt // 2:nt])
            w = nt // 4
            while w >= 1:
                nc.vector.tensor_max(
                    ctr[:, :, 0:w], ctr[:, :, 0:w], ctr[:, :, w:2 * w]
                )
                w //= 2
            cmask = fp.tile([P, nb, nt], dt.float16, tag="cmask")
            nc.vector.tensor_tensor(
                cmask[:], cm[:], ctr[:, :, 0:1].broadcast_to([P, nb, nt]),
                op=Alu.is_ge,
            )
            cw = fp.tile([P, nb, nt], dt.float32, tag="cw")
            nc.vector.tensor_tensor(cw[:], cmask[:], wtT[:], op=Alu.mult)
            w = nt // 2
            while w >= 2:
                nc.vector.tensor_add(
                    cw[:, :, 0:w], cw[:, :, 0:w], cw[:, :, w:2 * w]
                )
                w //= 2
            nc.vector.tensor_add(
                colres_sb[:].rearrange("p (b u) -> p b u", u=1),
                cw[:, :, 0:1], cw[:, :, 1:2],
            )
            nc.sync.dma_start(rowres_d[:, :], rowres_sb[:])
            nc.sync.dma_start(colres_d[:, :], colres_sb[:])

    nc.compile()
    return nc


def _f16(a):
    return a.astype(np.float16)


def _prep_inputs(x, y):
    """Host input marshalling: fp16 two-limb rows for the K=16 matmul.

    lhs rows (stationary, x side):  [xh0 xh0 xl0 xl0  xh1 xh1 xl1 xl1
                                     xh2 xh2 xl2 xl2  1 1  -sq1h -sq1l]
    rhs rows (moving, y side):      [yh0 yl0 yh0 yl0  yh1 yl1 yh1 yl1
                                     yh2 yl2 yh2 yl2  -sq2h -sq2l  1 1]
    with yh/yl the limbs of 2*y, so sum_k lhs[k]*rhs[k] = 2 x.y - sq1 - sq2.
    """
    n = x.shape[0]
    m = y.shape[0]
    Y = 2.0 * y
    xh = _f16(x.T)                       # [3, n]
    xl = _f16(x.T - xh.astype(np.float32))
    yh = _f16(Y.T)                       # [3, m]
    yl = _f16(Y.T - yh.astype(np.float32))
    sq1 = (x * x).sum(1)
    sq2 = (y * y).sum(1)
    s1h = _f16(-sq1)
    s1l = _f16(-sq1 - s1h.astype(np.float32))
    s2h = _f16(-sq2)
    s2l = _f16(-sq2 - s2h.astype(np.float32))
    one_n = np.ones(n, np.float16)
    one_m = np.ones(m, np.float16)
    lhs = np.stack([
        xh[0], xh[0], xl[0], xl[0],
        xh[1], xh[1], xl[1], xl[1],
        xh[2], xh[2], xl[2], xl[2],
        one_n, one_n, s1h, s1l,
    ])
    rhs = np.stack([
        yh[0], yl[0], yh[0], yl[0],
        yh[1], yl[1], yh[1], yl[1],
        yh[2], yl[2], yh[2], yl[2],
        s2h, s2l, one_m, one_m,
    ])
    return np.ascontiguousarray(lhs), np.ascontiguousarray(rhs)


def _run(nc, xyz1, xyz2, n_cores, trace=False):
    from concourse import bass_utils

    in_maps = []
    for b in range(n_cores):
        lhs, rhs = _prep_inputs(
            xyz1[b].astype(np.float32), xyz2[b].astype(np.float32)
        )
        in_maps.append({"lhs": lhs, "rhs": rhs})
    res = bass_utils.run_bass_kernel_spmd(
        nc, in_maps, core_ids=list(range(n_cores)), trace=trace,
    )
    return res


def _decode_ids(S, lim):
    """S = OFF*cnt + sum(winner ids). Returns (ids, bad) with ids valid
    where cnt==1 (S in [OFF, OFF+lim))."""
    r = np.rint(S).astype(np.int64)
    ids = r - int(OFF)
    bad = (ids < 0) | (ids >= lim) | (np.abs(S - r) > 0.25)
    return np.clip(ids, 0, lim - 1), bad


def _decode_batch(x, y, rowres, colres):
    """Exact fp32 refinement of device block/tile winners."""
    n, m = x.shape[0], y.shape[0]
    sq1 = (x * x).sum(1)
    sq2 = (y * y).sum(1)
    k = np.arange(P)

    # rows: i = 128*t + p  ->  rowres[p, t]
    S1 = rowres.T.reshape(-1)           # [n]
    g, bad1 = _decode_ids(S1, m // P)
    cols = g[:, None] * P + k[None, :]          # [n, P]
    db = sq1[:, None] + sq2[cols] - 2.0 * np.einsum(
        "nd,nkd->nk", x, y[cols], optimize=True)
    idx1 = np.take_along_axis(cols, db.argmin(1)[:, None], 1)[:, 0]
    if bad1.any():
        rows = np.nonzero(bad1)[0]
        dfull = sq1[rows, None] + sq2[None, :] - 2.0 * (x[rows] @ y.T)
        idx1[rows] = dfull.argmin(1)

    # cols: j = 128*b + jj  ->  colres[jj, b]
    S2 = colres.T.reshape(-1)           # [m]
    tt, bad2 = _decode_ids(S2, n // P)
    rows2 = tt[:, None] * P + k[None, :]        # [m, P]
    db2 = sq2[:, None] + sq1[rows2] - 2.0 * np.einsum(
        "md,mkd->mk", y, x[rows2], optimize=True)
    idx2 = np.take_along_axis(rows2, db2.argmin(1)[:, None], 1)[:, 0]
    if bad2.any():
        cls = np.nonzero(bad2)[0]
        dfull = sq2[cls, None] + sq1[None, :] - 2.0 * (y[cls] @ x.T)
        idx2[cls] = dfull.argmin(1)

    dist1 = sq1 + sq2[idx1] - 2.0 * (x * y[idx1]).sum(1)
    dist2 = sq2 + sq1[idx2] - 2.0 * (y * x[idx2]).sum(1)
    return (dist1.astype(np.float32), dist2.astype(np.float32),
            idx1.astype(np.int32), idx2.astype(np.int32))


def kernel(xyz1, xyz2, trace=False, _return_res=False):
    xyz1 = np.asarray(xyz1)
    xyz2 = np.asarray(xyz2)
    b, n, _ = xyz1.shape
    m = xyz2.shape[1]
    key = (n, m, b)
    if key not in _cache:
        _cache[key] = build(n=n, m=m, n_cores=b)
    nc = _cache[key]
    res = _run(nc, xyz1, xyz2, b, trace=trace)

    d1l, d2l, i1l, i2l = [], [], [], []
    for bb, r in enumerate(res.results):
        d1, d2, i1, i2 = _decode_batch(
            xyz1[bb].astype(np.float32), xyz2[bb].astype(np.float32),
            np.asarray(r["rowres"]), np.asarray(r["colres"]),
        )
        d1l.append(d1)
        d2l.append(d2)
        i1l.append(i1)
        i2l.append(i2)
    out = (np.stack(d1l), np.stack(d2l), np.stack(i1l), np.stack(i2l))
    if _return_res:
        return out, res
    return out


if __name__ == "__main__":
    rng = np.random.default_rng(0)
    x = rng.standard_normal((8, N, 3), dtype=np.float32)
    y = rng.standard_normal((8, M, 3), dtype=np.float32)
    d1, d2, i1, i2 = kernel(x, y)
    print("ok", d1.shape, d2.shape, i1.shape, i2.shape)



# revision 6
# speedup vs baseline: 5.1432x; 5.1432x over previous
"""Banded Chamfer-distance kernel for Trainium2 (8 NeuronCores, one batch/core).

Problem: xyz1, xyz2: [8, 8192, 3] fp32.  Per batch b:
  d[i,j] = |x_i|^2 + |y_j|^2 - 2 x_i.y_j
  dist1[i] = min_j d[i,j]; idx1[i] = argmin_j; dist2/idx2 symmetric.

Design (retrieval-KNN banding).  The host sorts BOTH clouds by the x
coordinate; nearest neighbours are then concentrated near the diagonal of
the (sorted) distance matrix.  The device computes only a static diagonal
BAND of W = 128*B columns per 128-row tile (s_t = clip(128t - W/2 + 64)),
an 8x work reduction at B=8, and reports
  - per-row per-16-column-group maxes of e = -d   (row path), and
  - per-tile per-column maxes of e                 (col path)
in fp16.  The host picks the winning group / tile per row / column,
refines exactly in fp64 within that small window, and checks a geometric
CERTIFICATE: because slabs are disjoint in x, any out-of-band point is at
least (x_i - band_edge_x)^2 away; if the refined best distance exceeds
that bound the row/col falls back to an exact host search (~0.2% of
points for gaussian clouds).  Results are exact up to fp16 group-argmax
near-ties (same noise class as the reference's own fp32 ties).

Device per tile t (band cols [s_t, s_t+W)):
  - PE: e = 2 x.y - sq1 - sq2 via the K=16 fp16 two-limb matmul (host
    marshals limbs), 2 x 512-wide matmuls into PSUM [128, 1024] fp32.
  - ACT: downconvert PSUM -> fp16 into eA [128, F] (pool-path cols) and
    the contiguous per-quad buffer eB [128, 4*TWB] (transpose-path cols).
  - Row path (DVE): halving max + tensor_reduce -> 16-col-group maxes
    into rowg[:, t, :].
  - Col path A (Pool): partition_all_reduce max over eA -> px; one DMA
    per quad of 4 tiles stores px rows straight to HBM.
  - Col path B (DMA+DVE): one batched xbar transpose per quad of eB ->
    et[jj, blk, i]; halving max + tensor_reduce -> cmT[:, t, blk].
"""

import numpy as np

B = 8
N = 8192   # rows per batch (xyz1 points)
M = 8192   # cols per batch (xyz2 points)
P = 128
K = 16     # matmul contraction rows (limb pairs + sq rows)
N_CORES = 8

NB = 8          # band width in 128-col blocks
W = NB * P      # 1024 band columns per row tile
F = 768         # pool-path columns (per tile); rest go through transpose
TW = W - F      # transpose-path columns (256 -> 2 blocks)
G = 16          # row-group width (host refine window)
QUAD = 4        # tiles batched per transpose / px-store DMA

_cache = {}


def band_start(t, n=N):
    return int(np.clip(128 * t - W // 2 + 64, 0, n - W))


def build(n=N, m=M, n_cores=N_CORES):
    """Build the Bass program. Returns the compiled Bacc object."""
    import concourse.bacc as bacc
    import concourse.tile as tile
    import concourse.mybir as mybir
    from concourse.bass_isa import ReduceOp

    dt = mybir.dt
    Alu = mybir.AluOpType

    nt = n // P   # row tiles
    assert nt % QUAD == 0
    ngA = F // G           # groups in pool part (48)
    ngB = TW // G          # groups in transpose part (16)
    ng = ngA + ngB         # 64 groups per tile
    nbB = TW // P          # transpose-path blocks per tile (2)

    nc = bacc.Bacc(
        "TRN2",
        target_bir_lowering=False,
        debug=False,
        enable_asserts=False,
        num_devices=n_cores,
    )

    lhs_d = nc.dram_tensor("lhs", [K, n], dt.float16, kind="ExternalInput").ap()
    rhs_d = nc.dram_tensor("rhs", [K, m], dt.float16, kind="ExternalInput").ap()
    rowg_d = nc.dram_tensor("rowg", [P, nt, ng], dt.float16,
                            kind="ExternalOutput").ap()
    colp_d = nc.dram_tensor("colp", [nt, F], dt.float16,
                            kind="ExternalOutput").ap()
    cmt_d = nc.dram_tensor("cmt", [P, nt, nbB], dt.float16,
                           kind="ExternalOutput").ap()

    with tile.TileContext(nc) as tc, tc.tile_pool(name="persist", bufs=1) as pp:
        lhs = pp.tile([K, n], dt.float16, name="lhs")
        rhs = pp.tile([K, m], dt.float16, name="rhs")
        # load what tile 0 needs first so its matmuls start early
        nc.sync.dma_start(lhs[:, 0:P], lhs_d[:, 0:P])
        nc.sync.dma_start(rhs[:, 0:W], rhs_d[:, 0:W])
        nc.sync.dma_start(rhs[:, W:m], rhs_d[:, W:m])
        nc.sync.dma_start(lhs[:, P:n], lhs_d[:, P:n])

        rowg = pp.tile([P, nt, ng], dt.float16, name="rowg")
        cmt = pp.tile([P, nt, nbB], dt.float16, name="cmt")

        with (
            tc.tile_pool(name="psum_e", bufs=3, space="PSUM") as psum_e,
            tc.tile_pool(name="ea_pool", bufs=3) as ea_pool,
            tc.tile_pool(name="eb_pool", bufs=2) as eb_pool,
            tc.tile_pool(name="et_pool", bufs=2) as et_pool,
            tc.tile_pool(name="px_pool", bufs=2) as px_pool,
            tc.tile_pool(name="scr_pool", bufs=3) as scr_pool,
        ):
            for q in range(nt // QUAD):
                eb = eb_pool.tile([P, QUAD * TW], dt.float16, tag="eb")
                px = px_pool.tile([P, QUAD * F], dt.float16, tag="px")
                for u in range(QUAD):
                    t = q * QUAD + u
                    s = band_start(t, m)
                    ps = psum_e.tile([P, W], dt.float32, tag="ps")
                    for h in range(2):
                        nc.tensor.matmul(
                            ps[:, 512 * h:512 * (h + 1)],
                            lhs[:, t * P:(t + 1) * P],
                            rhs[:, s + 512 * h: s + 512 * (h + 1)],
                            start=True, stop=True,
                        )
                    ea = ea_pool.tile([P, F], dt.float16, tag="ea")
                    nc.scalar.copy(ea[:], ps[:, 0:F])
                    nc.scalar.copy(eb[:, u * TW:(u + 1) * TW], ps[:, F:W])

                    # row path: halving max + reduce -> 16-col-group maxes
                    sA = scr_pool.tile([P, ngA, G // 2], dt.float16, tag="sA")
                    e3 = ea[:].rearrange("p (g i) -> p g i", g=ngA)
                    nc.vector.tensor_max(
                        sA[:], e3[:, :, 0:G // 2], e3[:, :, G // 2:G]
                    )
                    nc.vector.tensor_reduce(
                        rowg[:, t:t + 1, 0:ngA].rearrange("p a (g u) -> p (a g) u", u=1),
                        sA[:], axis=mybir.AxisListType.X, op=Alu.max,
                    )
                    sB = scr_pool.tile([P, ngB, G // 2], dt.float16, tag="sB")
                    b3 = eb[:, u * TW:(u + 1) * TW].rearrange(
                        "p (g i) -> p g i", g=ngB)
                    nc.vector.tensor_max(
                        sB[:], b3[:, :, 0:G // 2], b3[:, :, G // 2:G]
                    )
                    nc.vector.tensor_reduce(
                        rowg[:, t:t + 1, ngA:ng].rearrange("p a (g u) -> p (a g) u", u=1),
                        sB[:], axis=mybir.AxisListType.X, op=Alu.max,
                    )

                    # col path A: partition max of pool columns
                    nc.gpsimd.partition_all_reduce(
                        px[:, u * F:(u + 1) * F], ea[:], P, ReduceOp.max
                    )

                # col path A store: one DMA per quad (rows are replicated,
                # row 0 carries the result; dram rows are contiguous)
                nc.sync.dma_start(
                    colp_d[q * QUAD:(q + 1) * QUAD, :].rearrange(
                        "(u t) f -> u (t f)", u=1),
                    px[0:1, :],
                )

                # col path B: one batched transpose per quad + i-max tree
                et = et_pool.tile([P, QUAD * nbB, P], dt.float16, tag="et")
                nc.sync.dma_start(et[:], eb[:], transpose=True)
                sc = scr_pool.tile([P, QUAD * nbB, P // 2], dt.float16,
                                   tag="sc")
                nc.vector.tensor_max(
                    sc[:], et[:, :, 0:P // 2], et[:, :, P // 2:P]
                )
                nc.vector.tensor_reduce(
                    cmt[:, q * QUAD:(q + 1) * QUAD, :].rearrange(
                        "p t (b u) -> p (t b) u", u=1),
                    sc[:], axis=mybir.AxisListType.X, op=Alu.max,
                )

        with tc.tile_pool(name="fin_pool", bufs=1):
            nc.sync.dma_start(rowg_d[:, :, :], rowg[:])
            nc.sync.dma_start(cmt_d[:, :, :], cmt[:])

    nc.compile()
    return nc


def _f16(a):
    return a.astype(np.float16)


def _prep_inputs(x, y):
    """Host input marshalling: fp16 two-limb rows for the K=16 matmul.

    sum_k lhs[k,i]*rhs[k,j] = 2 x_i.y_j - sq1_i - sq2_j  (= -d[i,j])
    """
    n = x.shape[0]
    m = y.shape[0]
    Y = 2.0 * y
    xh = _f16(x.T)                       # [3, n]
    xl = _f16(x.T - xh.astype(np.float32))
    yh = _f16(Y.T)                       # [3, m]
    yl = _f16(Y.T - yh.astype(np.float32))
    sq1 = (x * x).sum(1)
    sq2 = (y * y).sum(1)
    s1h = _f16(-sq1)
    s1l = _f16(-sq1 - s1h.astype(np.float32))
    s2h = _f16(-sq2)
    s2l = _f16(-sq2 - s2h.astype(np.float32))
    one_n = np.ones(n, np.float16)
    one_m = np.ones(m, np.float16)
    lhs = np.stack([
        xh[0], xh[0], xl[0], xl[0],
        xh[1], xh[1], xl[1], xl[1],
        xh[2], xh[2], xl[2], xl[2],
        one_n, one_n, s1h, s1l,
    ])
    rhs = np.stack([
        yh[0], yl[0], yh[0], yl[0],
        yh[1], yl[1], yh[1], yl[1],
        yh[2], yl[2], yh[2], yl[2],
        s2h, s2l, one_m, one_m,
    ])
    return np.ascontiguousarray(lhs), np.ascontiguousarray(rhs)


def _run(nc, in_maps, n_cores, trace=False):
    from concourse import bass_utils

    res = bass_utils.run_bass_kernel_spmd(
        nc, in_maps, core_ids=list(range(n_cores)), trace=trace,
    )
    return res


def _decode_batch(xs, ys, rowg, colp, cmt):
    """Decode device outputs for one (sorted) batch.

    xs, ys: sorted points [N,3] float64. rowg [P, nt, ng] fp16,
    colp [nt, F] fp16, cmt [P, nt, nbB] fp16.
    Returns dist1, idx1, dist2, idx2 in SORTED coordinates.
    """
    n = xs.shape[0]
    m = ys.shape[0]
    nt = n // P
    ng = W // G
    sq1 = (xs * xs).sum(1)
    sq2 = (ys * ys).sum(1)
    s_t = np.array([band_start(t, m) for t in range(nt)])  # per row tile

    # ---- rows ----
    ii = np.arange(n)
    t_i = ii // P
    p_i = ii % P
    g = rowg[p_i, t_i, :].astype(np.float32).argmax(1)        # [n]
    w0 = s_t[t_i] + G * g
    cols = w0[:, None] + np.arange(G)[None, :]                # [n, G]
    d = (sq1[:, None] + sq2[cols]
         - 2.0 * np.einsum("nd,nkd->nk", xs, ys[cols], optimize=True))
    kk = d.argmin(1)
    idx1 = cols[ii, kk]
    dist1 = d[ii, kk]
    # certificate: out-of-band y's are at least this far (slabs in x)
    lo_edge = np.where(s_t > 0, ys[np.maximum(s_t - 1, 0), 0], -np.inf)
    hi_edge = np.where(s_t + W < m, ys[np.minimum(s_t + W, m - 1), 0], np.inf)
    rs = np.minimum((xs[:, 0] - lo_edge[t_i]) ** 2,
                    (hi_edge[t_i] - xs[:, 0]) ** 2)
    fb = dist1 > rs
    if fb.any():
        rows = np.nonzero(fb)[0]
        dfull = (sq1[rows, None] + sq2[None, :] - 2.0 * (xs[rows] @ ys.T))
        idx1[rows] = dfull.argmin(1)
        dist1[rows] = dfull.min(1)

    # ---- cols ----
    # coverage: tiles t with s_t <= j < s_t + W
    jj = np.arange(m)
    # candidate tiles: generous range, mask invalid
    t0 = np.clip((jj - W) // P + 1, 0, nt - 1)
    cand = t0[:, None] + np.arange(2 * NB + 2)[None, :]       # [m, 18]
    cand = np.clip(cand, 0, nt - 1)
    off = jj[:, None] - s_t[cand]                             # [m, 18]
    valid = (off >= 0) & (off < W)
    # value per (j, tile): pool part or transpose part
    offc = np.clip(off, 0, W - 1)
    pool_part = offc < F
    vals = np.full(cand.shape, -np.inf, np.float32)
    # pool values
    pv = colp[np.clip(cand, 0, nt - 1), np.clip(offc, 0, F - 1)]
    vals = np.where(valid & pool_part, pv.astype(np.float32), vals)
    # transpose values: blk = (off-F)//P, jloc = (off-F)%P
    blk = np.clip((offc - F) // P, 0, TW // P - 1)
    jloc = np.clip(offc - F, 0, TW - 1) % P
    tv = cmt[jloc, np.clip(cand, 0, nt - 1), blk]
    vals = np.where(valid & ~pool_part, tv.astype(np.float32), vals)
    tstar = cand[jj, vals.argmax(1)]                          # [m]
    rows2 = tstar[:, None] * P + np.arange(P)[None, :]        # [m, P]
    d2 = (sq2[:, None] + sq1[rows2]
          - 2.0 * np.einsum("md,mkd->mk", ys, xs[rows2], optimize=True))
    kk2 = d2.argmin(1)
    idx2 = rows2[jj, kk2]
    dist2 = d2[jj, kk2]
    # certificate: coverage edges in x for each column
    cov_lo = np.where(valid, s_t[cand] * 0 + cand * P, n)       # tile row start
    cov_hi = np.where(valid, cand * P + P, -1)
    lo_i = cov_lo.min(1)                                       # first covered row
    hi_i = cov_hi.max(1)                                       # one past last
    lo_e2 = np.where(lo_i > 0, xs[np.maximum(lo_i - 1, 0), 0], -np.inf)
    hi_e2 = np.where(hi_i < n, xs[np.minimum(hi_i, n - 1), 0], np.inf)
    rs2 = np.minimum((ys[:, 0] - lo_e2) ** 2, (hi_e2 - ys[:, 0]) ** 2)
    fb2 = dist2 > rs2
    if fb2.any():
        cls = np.nonzero(fb2)[0]
        dfull = (sq2[cls, None] + sq1[None, :] - 2.0 * (ys[cls] @ xs.T))
        idx2[cls] = dfull.argmin(1)
        dist2[cls] = dfull.min(1)

    return dist1, idx1, dist2, idx2


def kernel(xyz1, xyz2, trace=False, _return_res=False):
    xyz1 = np.asarray(xyz1)
    xyz2 = np.asarray(xyz2)
    b, n, _ = xyz1.shape
    m = xyz2.shape[1]
    key = (n, m, b)
    if key not in _cache:
        _cache[key] = build(n=n, m=m, n_cores=b)
    nc = _cache[key]

    # host prep: sort both clouds by x, marshal limbs
    orders = []
    in_maps = []
    for bb in range(b):
        x = xyz1[bb].astype(np.float32)
        y = xyz2[bb].astype(np.float32)
        o1 = np.argsort(x[:, 0], kind="stable")
        o2 = np.argsort(y[:, 0], kind="stable")
        orders.append((o1, o2))
        lhs, rhs = _prep_inputs(x[o1], y[o2])
        in_maps.append({"lhs": lhs, "rhs": rhs})

    res = _run(nc, in_maps, b, trace=trace)

    d1l, d2l, i1l, i2l = [], [], [], []
    for bb, r in enumerate(res.results):
        o1, o2 = orders[bb]
        xs = xyz1[bb].astype(np.float64)[o1]
        ys = xyz2[bb].astype(np.float64)[o2]
        d1s, i1s, d2s, i2s = _decode_batch(
            xs, ys,
            np.asarray(r["rowg"]), np.asarray(r["colp"]), np.asarray(r["cmt"]),
        )
        # map back to original indexing
        dist1 = np.empty(n, np.float32)
        idx1 = np.empty(n, np.int32)
        dist1[o1] = d1s.astype(np.float32)
        idx1[o1] = o2[i1s].astype(np.int32)
        dist2 = np.empty(m, np.float32)
        idx2 = np.empty(m, np.int32)
        dist2[o2] = d2s.astype(np.float32)
        idx2[o2] = o1[i2s].astype(np.int32)
        d1l.append(dist1)
        d2l.append(dist2)
        i1l.append(idx1)
        i2l.append(idx2)
    out = (np.stack(d1l), np.stack(d2l), np.stack(i1l), np.stack(i2l))
    if _return_res:
        return out, res
    return out


if __name__ == "__main__":
    rng = np.random.default_rng(0)
    x = rng.standard_normal((8, N, 3), dtype=np.float32)
    y = rng.standard_normal((8, M, 3), dtype=np.float32)
    d1, d2, i1, i2 = kernel(x, y)
    print("ok", d1.shape, d2.shape, i1.shape, i2.shape)
